# revision 4
# baseline (speedup 1.0000x reference)
"""Trainium2 Bass kernel for nn_EntityEncoder (gnn_message_passing).

Contract: kernel(**inputs) takes the FULL unsharded inputs (numpy) and
returns the full outputs (context_entity_hidden [32,48,128],
kb_entity_hidden [32,512,128]) as a tuple, matching reference().

Strategy: data-parallel over the batch dim (4 batches per NeuronCore,
8 cores, one SPMD program). Gathers run on-device (indirect DMA /
onehot matmuls); the per-edge relation matvec uses a relation-sorted
slot layout with a per-core slot->weight table so the instruction
stream is identical on every core.
"""
import sys

sys.path.insert(0, "/opt/trn_rl_repo")

from contextlib import ExitStack

import numpy as np

import concourse.bass as bass
import concourse.tile as tile
from concourse import bacc, mybir
from concourse.bass_utils import run_bass_kernel_spmd
from concourse.masks import make_identity

# problem shapes (hardcoded per spec)
B, L, EC, N, M, D, R, V = 32, 128, 48, 256, 512, 128, 100, 40000
NCORES = 8
BPC = B // NCORES          # batches per core = 4
SLOT = 32                  # edges per matvec slot (PE col-group width)
EDG = BPC * M              # edges per core = 2048
NE = BPC * N               # entities per core = 1024
F32 = mybir.dt.float32
I32 = mybir.dt.int32


# ---------------------------------------------------------------- host prep

def _host_prep(inputs):
    ce_emb = np.asarray(inputs["context_emb"], np.float32)
    ce_out = np.asarray(inputs["context_outputs"], np.float32)
    cmask = np.asarray(inputs["context_mask"], np.int32)
    cpos = np.asarray(inputs["context_entity_pos"], np.int32)
    cemask = np.asarray(inputs["context_entity_mask"], np.int32)
    entity = np.asarray(inputs["entity"], np.int32)
    kbe = np.asarray(inputs["kb_entity"], np.int32)
    kbm = np.asarray(inputs["kb_entity_mask"], np.int32)
    kbc = np.asarray(inputs["kb_entity_col"], np.int32)
    nei = np.asarray(inputs["kb_entity_nei"], np.int32)
    embed_table = np.asarray(inputs["embed_table"], np.float32)
    mlp1_w = np.asarray(inputs["mlp1_w"], np.float32)
    mlp1_b = np.asarray(inputs["mlp1_b"], np.float32)
    mlp2_w = np.asarray(inputs["mlp2_w"], np.float32)
    mlp2_b = np.asarray(inputs["mlp2_b"], np.float32)
    attn_wq = np.asarray(inputs["attn_wq"], np.float32)
    attn_bq = np.asarray(inputs["attn_bq"], np.float32)
    W = np.asarray(inputs["W"], np.float32)
    W0_w = np.asarray(inputs["W0_w"], np.float32)

    W_T = np.ascontiguousarray(W.transpose(0, 2, 1))  # [R, j, i] = W[r][i, j]

    # per-core relation slotting: edges sorted by (rel, b, m), chunked to 32
    per_core = []
    nslot_need = 0
    for c in range(NCORES):
        sl = slice(BPC * c, BPC * (c + 1))
        col = kbc[sl]
        bb = np.repeat(np.arange(BPC), M)
        mm = np.tile(np.arange(M), BPC)
        rr = col.ravel()
        order = np.lexsort((mm, bb, rr))  # sorted by rr, then bb, then mm
        slots = []  # (rel, [edge flat ids b*M+m])
        i = 0
        while i < EDG:
            r = rr[order[i]]
            j = i
            while j < EDG and rr[order[j]] == r:
                j += 1
            for k in range(i, j, SLOT):
                slots.append((int(r), order[k:min(k + SLOT, j)]))
            i = j
        per_core.append((sl, slots))
        nslot_need = max(nslot_need, len(slots))
    nslot = -(-nslot_need // 4) * 4  # multiple of 4 -> whole 128-row tiles
    nt = nslot * SLOT // 128         # number of 128-row tiles in sorted layout

    # mlp1_w.T is [2D, D] = [256, 128]; k-tile t = rows [128t:128t+128]
    # upload as [128, 2, 128] with [p, t, i] = mlp1_w.T[128t + p, i]
    m1 = mlp1_w.T.reshape(2, 128, 128).transpose(1, 0, 2)
    m2 = mlp2_w.T.reshape(2, 128, 128).transpose(1, 0, 2)
    shared = dict(
        mlp1_wt=np.ascontiguousarray(m1),
        mlp2_wt=np.ascontiguousarray(m2),
        mlp1_b=np.ascontiguousarray(mlp1_b.reshape(128, 1)),
        mlp2_b=np.ascontiguousarray(mlp2_b.reshape(128, 1)),
        wq_t=np.ascontiguousarray(attn_wq.T),
        bq=np.ascontiguousarray(attn_bq.reshape(128, 1)),
        w0_t=np.ascontiguousarray(W0_w.T),
        emb_tbl=embed_table,
    )

    in_maps = []
    for c in range(NCORES):
        sl, slots = per_core[c]
        # context / entity shards
        amask = np.where(cmask[sl] > 0, 0.0, -1e9).astype(np.float32)  # [4,128]
        oh1 = np.zeros((BPC, L, EC), np.float32)
        for b in range(BPC):
            oh1[b, cpos[sl][b], np.arange(EC)] = cemask[sl][b].astype(np.float32)
        ent_idx = np.ascontiguousarray(
            entity[sl].ravel().reshape(NE // 128, 128).T.astype(np.int32)
        )  # [128, 8]

        # sorted slot layout
        kbe_c, kbm_c = kbe[sl], kbm[sl]
        idx_s = np.zeros(nslot * SLOT, np.int32)
        msk_s = np.zeros(nslot * SLOT, np.float32)
        sc_i = (EDG + np.arange(nslot * SLOT) % 128).astype(np.int32)  # dummy rows
        wsel = np.zeros((nslot, 128, 128), np.float32)
        for s, (r, edges) in enumerate(slots):
            wsel[s] = W_T[r]
            pos = SLOT * s + np.arange(len(edges))
            eb, em = edges // M, edges % M
            idx_s[pos] = eb * N + kbe_c[eb, em]
            msk_s[pos] = kbm_c[eb, em].astype(np.float32)
            sc_i[pos] = eb * M + em
        idx_sorted = np.ascontiguousarray(idx_s.reshape(nt, 128).T)
        msk_sorted = np.ascontiguousarray(msk_s.reshape(nt, 128).T)
        sc_idx = np.ascontiguousarray(sc_i.reshape(nt, 128).T)
        w_sel = np.ascontiguousarray(wsel.transpose(1, 0, 2))  # [128, nslot, 128]

        # onehot for the original-order kb gather (mask folded in)
        oho = np.zeros((BPC, N, M), np.float32)
        for b in range(BPC):
            oho[b, kbe_c[b], np.arange(M)] = kbm_c[b].astype(np.float32)
        oh_orig = np.ascontiguousarray(
            oho.reshape(BPC, 2, 128, M).transpose(0, 2, 1, 3)
        )  # [4, 128, 2, 512]

        # degree-normalized transposed neighbor matrix, original order
        nei_c = nei[sl].astype(np.float32)  # [4, M, M]
        deg = np.clip(nei_c.sum(axis=2), 1.0, None)  # [4, M]
        nn = nei_c / deg[:, :, None]                  # [4, M(m), M(n)]
        nnT = nn.transpose(0, 2, 1)                   # [4, n, m]
        nei_t = np.ascontiguousarray(
            nnT.reshape(BPC, 4, 128, M).transpose(0, 2, 1, 3).reshape(BPC, 128, 4 * M)
        )  # [4, 128, 2048]

        m = dict(shared)
        m.update(
            ce_emb=np.ascontiguousarray(ce_emb[sl]),
            ce_out=np.ascontiguousarray(ce_out[sl]),
            amask=amask,
            onehot1=np.ascontiguousarray(oh1.transpose(1, 0, 2)),  # [128, 4, 48]
            ent_idx=ent_idx,
            idx_sorted=idx_sorted,
            msk_sorted=msk_sorted,
            sc_idx=sc_idx,
            w_sel=w_sel,
            oh_orig=oh_orig,
            nei_t=nei_t,
        )
        in_maps.append(m)
    return in_maps, nslot, nt


# ------------------------------------------------------------- bass program

def _build_program(nslot, nt):
    nc = bacc.Bacc("TRN2", target_bir_lowering=False, debug=False,
                   num_devices=NCORES)

    def din(name, shape, dt=F32):
        return nc.dram_tensor(name, list(shape), dt, kind="ExternalInput").ap()

    ce_emb = din("ce_emb", (BPC, 128, 128))
    ce_out = din("ce_out", (BPC, 128, 128))
    amask = din("amask", (BPC, 128))
    onehot1 = din("onehot1", (128, BPC, EC))
    mlp1_wt = din("mlp1_wt", (128, 2, 128))
    mlp1_b = din("mlp1_b", (128, 1))
    mlp2_wt = din("mlp2_wt", (128, 2, 128))
    mlp2_b = din("mlp2_b", (128, 1))
    wq_t = din("wq_t", (128, 128))
    bq = din("bq", (128, 1))
    w0_t = din("w0_t", (128, 128))
    emb_tbl = din("emb_tbl", (V, 128))
    ent_idx = din("ent_idx", (128, NE // 128), I32)
    idx_sorted = din("idx_sorted", (128, nt), I32)
    msk_sorted = din("msk_sorted", (128, nt))
    sc_idx = din("sc_idx", (128, nt), I32)
    w_sel = din("w_sel", (128, nslot, 128))
    oh_orig = din("oh_orig", (BPC, 128, 2, M))
    nei_t = din("nei_t", (BPC, 128, 4 * M))

    out_ctx = nc.dram_tensor("out_ctx", [BPC * EC, 128], F32,
                             kind="ExternalOutput").ap()
    out_kb = nc.dram_tensor("out_kb", [EDG, 128], F32,
                            kind="ExternalOutput").ap()

    eh_rows = nc.dram_tensor("eh_rows", [NE, 128], F32).ap()
    ks_rows = nc.dram_tensor("ks_rows", [EDG + 128, 128], F32).ap()

    with tile.TileContext(nc) as tc, ExitStack() as ctx:
        consts = ctx.enter_context(tc.tile_pool(name="consts", bufs=1))
        big = ctx.enter_context(tc.tile_pool(name="big", bufs=1))
        work = ctx.enter_context(tc.tile_pool(name="work", bufs=3))
        keep = ctx.enter_context(tc.tile_pool(name="keep", bufs=1))
        small = ctx.enter_context(tc.tile_pool(name="small", bufs=4))

        # ---- resident constants / big inputs
        ident = consts.tile([128, 128], F32)
        make_identity(nc, ident[:])
        m1w = consts.tile([128, 2, 128], F32)
        nc.sync.dma_start(out=m1w[:], in_=mlp1_wt[:])
        m2w = consts.tile([128, 2, 128], F32)
        nc.sync.dma_start(out=m2w[:], in_=mlp2_wt[:])
        m1b = consts.tile([128, 1], F32)
        nc.sync.dma_start(out=m1b[:], in_=mlp1_b[:])
        m2b = consts.tile([128, 1], F32)
        nc.sync.dma_start(out=m2b[:], in_=mlp2_b[:])
        wq = consts.tile([128, 128], F32)
        nc.sync.dma_start(out=wq[:], in_=wq_t[:])
        bqs = consts.tile([128, 1], F32)
        nc.sync.dma_start(out=bqs[:], in_=bq[:])
        w0 = consts.tile([128, 128], F32)
        nc.sync.dma_start(out=w0[:], in_=w0_t[:])
        oh1 = consts.tile([128, BPC, EC], F32)
        nc.sync.dma_start(out=oh1[:], in_=onehot1[:])
        eidx = consts.tile([128, NE // 128], I32)
        nc.sync.dma_start(out=eidx[:], in_=ent_idx[:])
        sidx = consts.tile([128, nt], I32)
        nc.sync.dma_start(out=sidx[:], in_=idx_sorted[:])
        smsk = consts.tile([128, nt], F32)
        nc.sync.dma_start(out=smsk[:], in_=msk_sorted[:])
        scix = consts.tile([128, nt], I32)
        nc.sync.dma_start(out=scix[:], in_=sc_idx[:])

        cem = [consts.tile([128, 128], F32, name=f"cem{b}") for b in range(BPC)]
        ceo = [consts.tile([128, 128], F32, name=f"ceo{b}") for b in range(BPC)]
        for b in range(BPC):
            nc.sync.dma_start(out=cem[b][:], in_=ce_emb[b])
            nc.sync.dma_start(out=ceo[b][:], in_=ce_out[b])

        # big background loads (needed in phase C)
        wsel_sb = big.tile([128, nslot, 128], F32)
        for q in range(4):  # chunked so it interleaves with other traffic
            qs = nslot // 4
            nc.sync.dma_start(out=wsel_sb[:, q * qs:(q + 1) * qs, :],
                              in_=w_sel[:, q * qs:(q + 1) * qs, :])
        nei_sb = [big.tile([128, 4 * M], F32, name=f"nei{b}") for b in range(BPC)]
        for b in range(BPC):
            nc.sync.dma_start(out=nei_sb[b][:], in_=nei_t[b])
        oho_sb = [big.tile([128, 2, M], F32, name=f"oho{b}") for b in range(BPC)]
        for b in range(BPC):
            nc.sync.dma_start(out=oho_sb[b][:], in_=oh_orig[b])

        with tc.tile_pool(name="psA", bufs=3, space="PSUM") as psA, \
             tc.tile_pool(name="psB", bufs=4, space="PSUM") as psB:

            # ================= phase A: context-entity hidden =================
            cehT = [keep.tile([128, BPC * EC], F32, name=f"cehT{k}")
                    for k in range(2)]
            for b in range(BPC):
                for k, src in ((0, cem[b]), (1, ceo[b])):
                    aps = psA.tile([128, EC], F32, space="PSUM", tag="a")
                    nc.tensor.matmul(out=aps[:], lhsT=src[:],
                                     rhs=oh1[:, b, :], start=True, stop=True)
                    nc.scalar.copy(out=cehT[k][:, b * EC:(b + 1) * EC],
                                   in_=aps[:])
            o1ps = psB.tile([128, BPC * EC], F32, space="PSUM", tag="b")
            nc.tensor.matmul(out=o1ps[:], lhsT=m1w[:, 0, :], rhs=cehT[0][:],
                             start=True, stop=False)
            nc.tensor.matmul(out=o1ps[:], lhsT=m1w[:, 1, :], rhs=cehT[1][:],
                             start=False, stop=True)
            o1T = work.tile([128, BPC * EC], F32)
            nc.scalar.activation(out=o1T[:], in_=o1ps[:],
                                 func=mybir.ActivationFunctionType.Relu,
                                 bias=m1b[:, 0:1])
            for h in range(2):
                tp = psA.tile([96, 128], F32, space="PSUM", tag="a")
                nc.tensor.transpose(out=tp[:], in_=o1T[:, h * 96:(h + 1) * 96],
                                    identity=ident[:])
                o1r = work.tile([96, 128], F32)
                nc.vector.tensor_copy(out=o1r[:], in_=tp[:])
                nc.sync.dma_start(out=out_ctx[h * 96:(h + 1) * 96, :],
                                  in_=o1r[:])

            # ================= phase B: entity attention + mlp2 ==============
            ehr = [[keep.tile([128, 128], F32, name=f"ehr{b}_{j}")
                    for j in range(2)] for b in range(BPC)]
            for b in range(BPC):
                # context_emb transposed (scores rhs)
                tp = psA.tile([128, 128], F32, space="PSUM", tag="a")
                nc.tensor.transpose(out=tp[:], in_=cem[b][:], identity=ident[:])
                ceT = work.tile([128, 128], F32)
                nc.scalar.copy(out=ceT[:], in_=tp[:])

                # entity embedding gather -> emb_T [d, n]
                embT = work.tile([128, N], F32)
                for j in range(2):
                    er = small.tile([128, 128], F32)
                    nc.gpsimd.indirect_dma_start(
                        out=er[:], out_offset=None, in_=emb_tbl[:],
                        in_offset=bass.IndirectOffsetOnAxis(
                            ap=eidx[:, 2 * b + j:2 * b + j + 1], axis=0))
                    tp2 = psA.tile([128, 128], F32, space="PSUM", tag="a")
                    nc.tensor.transpose(out=tp2[:], in_=er[:],
                                        identity=ident[:])
                    nc.vector.tensor_copy(out=embT[:, j * 128:(j + 1) * 128],
                                          in_=tp2[:])

                # q_T = wq @ emb_T + bq
                qps = psB.tile([128, N], F32, space="PSUM", tag="b")
                nc.tensor.matmul(out=qps[:], lhsT=wq[:], rhs=embT[:],
                                 start=True, stop=True)
                qT = work.tile([128, N], F32)
                nc.scalar.activation(out=qT[:], in_=qps[:],
                                     func=mybir.ActivationFunctionType.Identity,
                                     bias=bqs[:, 0:1])

                # additive context mask, broadcast across partitions
                amb = small.tile([128, 128], F32)
                nc.sync.dma_start(
                    out=amb[:],
                    in_=bass.AP(tensor=amask.tensor, offset=b * 128,
                                ap=[[0, 128], [1, 128]]))

                awT = work.tile([128, N], F32)
                for ntl in range(2):
                    sps = psB.tile([128, 128], F32, space="PSUM", tag="b")
                    nc.tensor.matmul(out=sps[:],
                                     lhsT=qT[:, ntl * 128:(ntl + 1) * 128],
                                     rhs=ceT[:], start=True, stop=True)
                    sc = small.tile([128, 128], F32)
                    nc.vector.tensor_tensor(out=sc[:], in0=sps[:], in1=amb[:],
                                            op=mybir.AluOpType.add)
                    rmax = small.tile([128, 1], F32)
                    nc.vector.tensor_reduce(out=rmax[:], in_=sc[:],
                                            axis=mybir.AxisListType.X,
                                            op=mybir.AluOpType.max,
                                            negate=True)
                    sexp = small.tile([128, 1], F32)
                    nc.scalar.activation(out=sc[:], in_=sc[:],
                                         func=mybir.ActivationFunctionType.Exp,
                                         bias=rmax[:, 0:1],
                                         accum_out=sexp[:, 0:1])
                    rs = small.tile([128, 1], F32)
                    nc.vector.reciprocal(out=rs[:], in_=sexp[:])
                    nc.vector.tensor_scalar_mul(sc[:], sc[:], rs[:, 0:1])
                    tp3 = psA.tile([128, 128], F32, space="PSUM", tag="a")
                    nc.tensor.transpose(out=tp3[:], in_=sc[:],
                                        identity=ident[:])
                    nc.vector.tensor_copy(out=awT[:, ntl * 128:(ntl + 1) * 128],
                                          in_=tp3[:])

                # entity_align_T = ce_emb.T @ attw_T
                alps = psB.tile([128, N], F32, space="PSUM", tag="b")
                nc.tensor.matmul(out=alps[:], lhsT=cem[b][:], rhs=awT[:],
                                 start=True, stop=True)
                alT = work.tile([128, N], F32)
                nc.scalar.copy(out=alT[:], in_=alps[:])

                # entity_hidden_T = relu(mlp2_wT.T @ [emb_T; align_T] + b2)
                ehps = psB.tile([128, N], F32, space="PSUM", tag="b")
                nc.tensor.matmul(out=ehps[:], lhsT=m2w[:, 0, :], rhs=embT[:],
                                 start=True, stop=False)
                nc.tensor.matmul(out=ehps[:], lhsT=m2w[:, 1, :], rhs=alT[:],
                                 start=False, stop=True)
                ehT = work.tile([128, N], F32)
                nc.scalar.activation(out=ehT[:], in_=ehps[:],
                                     func=mybir.ActivationFunctionType.Relu,
                                     bias=m2b[:, 0:1])
                for j in range(2):
                    tp4 = psA.tile([128, 128], F32, space="PSUM", tag="a")
                    nc.tensor.transpose(out=tp4[:],
                                        in_=ehT[:, j * 128:(j + 1) * 128],
                                        identity=ident[:])
                    nc.vector.tensor_copy(out=ehr[b][j][:], in_=tp4[:])
                    nc.sync.dma_start(
                        out=eh_rows[b * N + j * 128:b * N + (j + 1) * 128, :],
                        in_=ehr[b][j][:])

        # ================= phase C: KB graph ============================
        with tc.tile_pool(name="psT", bufs=2, space="PSUM") as psT, \
             tc.tile_pool(name="psK", bufs=4, space="PSUM") as psK, \
             tc.tile_pool(name="psG", bufs=2, space="PSUM") as psG:

            # original-order kb_init (onehot matmul; mask folded in)
            kbiT = [keep.tile([128, M], F32, name=f"kbiT{b}")
                    for b in range(BPC)]
            for b in range(BPC):
                kps = psG.tile([128, M], F32, space="PSUM", tag="g")
                for kt in range(2):
                    nc.tensor.matmul(out=kps[:], lhsT=ehr[b][kt][:],
                                     rhs=oho_sb[b][:, kt, :],
                                     start=(kt == 0), stop=(kt == 1))
                nc.scalar.copy(out=kbiT[b][:], in_=kps[:])

            # sorted gather -> xs_T [128, nslot*32]
            xsT = big.tile([128, nslot * SLOT], F32)
            for t in range(nt):
                g = small.tile([128, 128], F32, name="gat")
                nc.gpsimd.indirect_dma_start(
                    out=g[:], out_offset=None, in_=eh_rows[:],
                    in_offset=bass.IndirectOffsetOnAxis(
                        ap=sidx[:, t:t + 1], axis=0))
                nc.vector.tensor_scalar_mul(g[:], g[:], smsk[:, t:t + 1])
                tps = psT.tile([128, 128], F32, space="PSUM", tag="t")
                nc.tensor.transpose(out=tps[:], in_=g[:], identity=ident[:])
                nc.vector.tensor_copy(out=xsT[:, t * 128:(t + 1) * 128],
                                      in_=tps[:])

            # relation matvec: slot s -> ks rows [32s, 32s+32)
            for t in range(nt):
                kp = psK.tile([128, 128], F32, space="PSUM", tag="k")
                for q in range(4):
                    s = 4 * t + q
                    ro = SLOT * q
                    nc.tensor.matmul(
                        out=kp[ro:ro + SLOT, :],
                        lhsT=xsT[:, s * SLOT:(s + 1) * SLOT],
                        rhs=wsel_sb[:, s, :],
                        start=True, stop=True,
                        tile_position=(0, ro))
                kr = small.tile([128, 128], F32, name="krow")
                nc.scalar.copy(out=kr[:], in_=kp[:])
                nc.gpsimd.indirect_dma_start(
                    out=ks_rows[:], in_=kr[:],
                    out_offset=bass.IndirectOffsetOnAxis(
                        ap=scix[:, t:t + 1], axis=0),
                    in_offset=None)

            # aggregation: kb_hidden_T = relu(W0 @ kb_init_T + nei_norm_T-sum)
            for b in range(BPC):
                ksb = work.tile([128, 4, 128], F32, name="ksb")
                nc.sync.dma_start(
                    out=ksb[:],
                    in_=ks_rows[b * M:(b + 1) * M, :].rearrange(
                        "(kt p) d -> p kt d", p=128))
                agg = psG.tile([128, M], F32, space="PSUM", tag="g")
                nc.tensor.matmul(out=agg[:], lhsT=w0[:], rhs=kbiT[b][:],
                                 start=True, stop=False)
                for kt in range(4):
                    nc.tensor.matmul(out=agg[:], lhsT=ksb[:, kt, :],
                                     rhs=nei_sb[b][:, kt * M:(kt + 1) * M],
                                     start=False, stop=(kt == 3))
                khT = work.tile([128, M], F32, name="khT")
                nc.scalar.activation(out=khT[:], in_=agg[:],
                                     func=mybir.ActivationFunctionType.Relu)
                for kt in range(4):
                    tpo = psT.tile([128, 128], F32, space="PSUM", tag="t")
                    nc.tensor.transpose(out=tpo[:],
                                        in_=khT[:, kt * 128:(kt + 1) * 128],
                                        identity=ident[:])
                    orow = small.tile([128, 128], F32, name="orow")
                    nc.vector.tensor_copy(out=orow[:], in_=tpo[:])
                    nc.sync.dma_start(
                        out=out_kb[b * M + kt * 128:b * M + (kt + 1) * 128, :],
                        in_=orow[:])

    nc.compile()
    return nc


_CACHE = {}


def _get_program(nslot, nt):
    key = (nslot, nt)
    if key not in _CACHE:
        _CACHE[key] = _build_program(nslot, nt)
    return _CACHE[key]


def kernel(**inputs):
    in_maps, nslot, nt = _host_prep(inputs)
    nc = _get_program(nslot, nt)
    res = run_bass_kernel_spmd(nc, in_maps, list(range(NCORES)))
    out_ctx = np.concatenate(
        [res.results[c]["out_ctx"].reshape(BPC, EC, 128) for c in range(NCORES)])
    out_kb = np.concatenate(
        [res.results[c]["out_kb"].reshape(BPC, M, 128) for c in range(NCORES)])
    return out_ctx, out_kb


# revision 6
# speedup vs baseline: 1.3496x; 1.3496x over previous
"""Trainium2 Bass kernel for nn_EntityEncoder (gnn_message_passing).

Contract: kernel(**inputs) takes the FULL unsharded inputs (numpy) and
returns the full outputs (context_entity_hidden [32,48,128],
kb_entity_hidden [32,512,128]) as a tuple, matching reference().

Strategy: data-parallel over the batch dim (4 batches per NeuronCore,
8 cores, one SPMD program). Gathers run on-device (indirect DMA /
onehot matmuls); the per-edge relation matvec uses a relation-sorted
32-wide slot layout with a per-core slot->weight table so the
instruction stream is identical on every core. kb_state rows are
stored sorted and un-sorted by an inverse-permutation gather feeding a
row-oriented PSUM-accumulated aggregation.
"""
import sys

sys.path.insert(0, "/opt/trn_rl_repo")

from contextlib import ExitStack

import numpy as np

import concourse.bass as bass
import concourse.tile as tile
from concourse import bacc, mybir
from concourse.bass_utils import run_bass_kernel_spmd
from concourse.masks import make_identity

# problem shapes (hardcoded per spec)
B, L, EC, N, M, D, R, V = 32, 128, 48, 256, 512, 128, 100, 40000
NCORES = 8
BPC = B // NCORES          # batches per core = 4
SLOT = 32                  # edges per matvec slot (PE col-group width)
EDG = BPC * M              # edges per core = 2048
NE = BPC * N               # entities per core = 1024
F32 = mybir.dt.float32
I32 = mybir.dt.int32


# ---------------------------------------------------------------- host prep

def _host_prep(inputs):
    ce_emb = np.asarray(inputs["context_emb"], np.float32)
    ce_out = np.asarray(inputs["context_outputs"], np.float32)
    cmask = np.asarray(inputs["context_mask"], np.int32)
    cpos = np.asarray(inputs["context_entity_pos"], np.int32)
    cemask = np.asarray(inputs["context_entity_mask"], np.int32)
    entity = np.asarray(inputs["entity"], np.int32)
    kbe = np.asarray(inputs["kb_entity"], np.int32)
    kbm = np.asarray(inputs["kb_entity_mask"], np.int32)
    kbc = np.asarray(inputs["kb_entity_col"], np.int32)
    nei = np.asarray(inputs["kb_entity_nei"], np.int32)
    embed_table = np.asarray(inputs["embed_table"], np.float32)
    mlp1_w = np.asarray(inputs["mlp1_w"], np.float32)
    mlp1_b = np.asarray(inputs["mlp1_b"], np.float32)
    mlp2_w = np.asarray(inputs["mlp2_w"], np.float32)
    mlp2_b = np.asarray(inputs["mlp2_b"], np.float32)
    attn_wq = np.asarray(inputs["attn_wq"], np.float32)
    attn_bq = np.asarray(inputs["attn_bq"], np.float32)
    W = np.asarray(inputs["W"], np.float32)
    W0_w = np.asarray(inputs["W0_w"], np.float32)

    W_T = np.ascontiguousarray(W.transpose(0, 2, 1))  # [R, j, i] = W[r][i, j]

    # per-core relation slotting: edges sorted by (rel, b, m), chunked to 32
    per_core = []
    nslot_need = 0
    for c in range(NCORES):
        sl = slice(BPC * c, BPC * (c + 1))
        rr = kbc[sl].ravel()
        bb = np.repeat(np.arange(BPC), M)
        mm = np.tile(np.arange(M), BPC)
        order = np.lexsort((mm, bb, rr))  # sorted by rr, then bb, then mm
        slots = []  # (rel, [edge flat ids b*M+m])
        i = 0
        while i < EDG:
            r = rr[order[i]]
            j = i
            while j < EDG and rr[order[j]] == r:
                j += 1
            for k in range(i, j, SLOT):
                slots.append((int(r), order[k:min(k + SLOT, j)]))
            i = j
        per_core.append((sl, slots))
        nslot_need = max(nslot_need, len(slots))
    nslot = -(-nslot_need // 4) * 4  # multiple of 4 -> whole 128-row tiles
    nt = nslot * SLOT // 128         # tiles in sorted layout
    nto = EDG // 128                 # tiles in original layout (16)

    # mlp*_w.T is [2D, D]; upload k-tiles as [128, 2, 128]
    m1 = mlp1_w.T.reshape(2, 128, 128).transpose(1, 0, 2)
    m2 = mlp2_w.T.reshape(2, 128, 128).transpose(1, 0, 2)
    shared = dict(
        mlp1_wt=np.ascontiguousarray(m1),
        mlp2_wt=np.ascontiguousarray(m2),
        mlp1_b=np.ascontiguousarray(mlp1_b.reshape(128, 1)),
        mlp2_b=np.ascontiguousarray(mlp2_b.reshape(128, 1)),
        wq_t=np.ascontiguousarray(attn_wq.T),
        bq=np.ascontiguousarray(attn_bq.reshape(128, 1)),
        w0_t=np.ascontiguousarray(W0_w.T),
        emb_tbl=embed_table,
    )

    in_maps = []
    for c in range(NCORES):
        sl, slots = per_core[c]
        amask = np.where(cmask[sl] > 0, 0.0, -1e9).astype(np.float32)  # [4,128]
        oh1 = np.zeros((BPC, L, EC), np.float32)
        for b in range(BPC):
            oh1[b, cpos[sl][b], np.arange(EC)] = cemask[sl][b].astype(np.float32)
        ent_idx = np.ascontiguousarray(
            entity[sl].ravel().reshape(NE // 128, 128).T.astype(np.int32))

        # sorted slot layout (gather indices + mask); inverse perm for unsort
        kbe_c, kbm_c = kbe[sl], kbm[sl]
        idx_s = np.zeros(nslot * SLOT, np.int32)
        msk_s = np.zeros(nslot * SLOT, np.float32)
        inv = np.zeros(EDG, np.int32)
        wsel = np.zeros((nslot, 128, 128), np.float32)
        for s, (r, edges) in enumerate(slots):
            wsel[s] = W_T[r]
            pos = SLOT * s + np.arange(len(edges))
            eb, em = edges // M, edges % M
            idx_s[pos] = eb * N + kbe_c[eb, em]
            msk_s[pos] = kbm_c[eb, em].astype(np.float32)
            inv[edges] = pos
        idx_sorted = np.ascontiguousarray(idx_s.reshape(nt, 128).T)
        msk_sorted = np.ascontiguousarray(msk_s.reshape(nt, 128).T)
        inv_idx = np.ascontiguousarray(inv.reshape(nto, 128).T)
        w_sel = np.ascontiguousarray(wsel.transpose(1, 0, 2))  # [128,nslot,128]

        # onehot for the original-order kb gather (mask folded in)
        oho = np.zeros((BPC, N, M), np.float32)
        for b in range(BPC):
            oho[b, kbe_c[b], np.arange(M)] = kbm_c[b].astype(np.float32)
        oh_orig = np.ascontiguousarray(
            oho.reshape(BPC, 2, 128, M).transpose(0, 2, 1, 3))  # [4,128,2,512]

        # degree-normalized transposed neighbor matrix, original order
        nei_c = nei[sl].astype(np.float32)            # [4, M, M]
        deg = np.clip(nei_c.sum(axis=2), 1.0, None)   # [4, M]
        nnT = (nei_c / deg[:, :, None]).transpose(0, 2, 1)  # [4, n, m]
        nei_t = np.ascontiguousarray(
            nnT.reshape(BPC, 4, 128, M).transpose(0, 2, 1, 3).reshape(
                BPC, 128, 4 * M))  # [4, 128, 2048]

        m = dict(shared)
        m.update(
            ce_emb=np.ascontiguousarray(ce_emb[sl]),
            ce_out=np.ascontiguousarray(ce_out[sl]),
            amask=amask,
            onehot1=np.ascontiguousarray(oh1.transpose(1, 0, 2)),  # [128,4,48]
            ent_idx=ent_idx,
            idx_sorted=idx_sorted,
            msk_sorted=msk_sorted,
            inv_idx=inv_idx,
            w_sel=w_sel,
            oh_orig=oh_orig,
            nei_t=nei_t,
        )
        in_maps.append(m)
    return in_maps, nslot, nt


# ------------------------------------------------------------- bass program

def _build_program(nslot, nt):
    nto = EDG // 128
    nc = bacc.Bacc("TRN2", target_bir_lowering=False, debug=False,
                   num_devices=NCORES)

    def din(name, shape, dt=F32):
        return nc.dram_tensor(name, list(shape), dt, kind="ExternalInput").ap()

    ce_emb = din("ce_emb", (BPC, 128, 128))
    ce_out = din("ce_out", (BPC, 128, 128))
    amask = din("amask", (BPC, 128))
    onehot1 = din("onehot1", (128, BPC, EC))
    mlp1_wt = din("mlp1_wt", (128, 2, 128))
    mlp1_b = din("mlp1_b", (128, 1))
    mlp2_wt = din("mlp2_wt", (128, 2, 128))
    mlp2_b = din("mlp2_b", (128, 1))
    wq_t = din("wq_t", (128, 128))
    bq = din("bq", (128, 1))
    w0_t = din("w0_t", (128, 128))
    emb_tbl = din("emb_tbl", (V, 128))
    ent_idx = din("ent_idx", (128, NE // 128), I32)
    idx_sorted = din("idx_sorted", (128, nt), I32)
    msk_sorted = din("msk_sorted", (128, nt))
    inv_idx = din("inv_idx", (128, nto), I32)
    w_sel = din("w_sel", (128, nslot, 128))
    oh_orig = din("oh_orig", (BPC, 128, 2, M))
    nei_t = din("nei_t", (BPC, 128, 4 * M))

    out_ctx = nc.dram_tensor("out_ctx", [BPC * EC, 128], F32,
                             kind="ExternalOutput").ap()
    out_kb = nc.dram_tensor("out_kb", [EDG, 128], F32,
                            kind="ExternalOutput").ap()

    eh_rows = nc.dram_tensor("eh_rows", [NE, 128], F32).ap()
    ks_sort = nc.dram_tensor("ks_sort", [nt * 128, 128], F32).ap()

    with tile.TileContext(nc) as tc, ExitStack() as ctx:
        consts = ctx.enter_context(tc.tile_pool(name="consts", bufs=1))
        big = ctx.enter_context(tc.tile_pool(name="big", bufs=1))
        work = ctx.enter_context(tc.tile_pool(name="work", bufs=3))
        keep = ctx.enter_context(tc.tile_pool(name="keep", bufs=1))
        small = ctx.enter_context(tc.tile_pool(name="small", bufs=4))

        # ---- gpsimd: identity first (all transposes need it)
        ident = consts.tile([128, 128], F32)
        make_identity(nc, ident[:])

        # ---- sync queue: small latency-critical loads first
        def ld(pool, shape, src, dt=F32, engine=None, name=None):
            t = pool.tile(shape, dt, name=name)
            (engine or nc.sync).dma_start(out=t[:], in_=src)
            return t

        m1w = ld(consts, [128, 2, 128], mlp1_wt[:], name="m1w")
        m2w = ld(consts, [128, 2, 128], mlp2_wt[:], name="m2w")
        m1b = ld(consts, [128, 1], mlp1_b[:], name="m1b")
        m2b = ld(consts, [128, 1], mlp2_b[:], name="m2b")
        wq = ld(consts, [128, 128], wq_t[:], name="wq")
        bqs = ld(consts, [128, 1], bq[:], name="bqs")
        w0 = ld(consts, [128, 128], w0_t[:], name="w0")
        oh1 = ld(consts, [128, BPC, EC], onehot1[:], name="oh1")
        eidx = ld(consts, [128, NE // 128], ent_idx[:], I32, name="eidx")
        sidx = ld(consts, [128, nt], idx_sorted[:], I32, name="sidx")
        smsk = ld(consts, [128, nt], msk_sorted[:], name="smsk")
        vidx = ld(consts, [128, nto], inv_idx[:], I32, name="vidx")
        amb = []
        for b in range(BPC):
            amb.append(ld(consts, [128, 128],
                          bass.AP(tensor=amask.tensor, offset=b * 128,
                                  ap=[[0, 128], [1, 128]]), name=f"amb{b}"))
        cem = [ld(consts, [128, 128], ce_emb[b], name=f"cem{b}")
               for b in range(BPC)]
        ceo = [ld(consts, [128, 128], ce_out[b], name=f"ceo{b}")
               for b in range(BPC)]

        # ---- gpsimd: embedding gathers early (phase B input)
        embr = [[small.tile([128, 128], F32, name=f"embr{b}_{j}", bufs=1)
                 for j in range(2)] for b in range(BPC)]
        for b in range(BPC):
            for j in range(2):
                nc.gpsimd.indirect_dma_start(
                    out=embr[b][j][:], out_offset=None, in_=emb_tbl[:],
                    in_offset=bass.IndirectOffsetOnAxis(
                        ap=eidx[:, 2 * b + j:2 * b + j + 1], axis=0))

        # ---- bulk background loads: w_sel on gpsimd, nei/oh on scalar
        wsel_sb = big.tile([128, nslot, 128], F32)
        qs = nslot // 4
        for q in range(4):
            nc.gpsimd.dma_start(out=wsel_sb[:, q * qs:(q + 1) * qs, :],
                                in_=w_sel[:, q * qs:(q + 1) * qs, :])
        nei_sb = [ld(big, [128, 4 * M], nei_t[b], engine=nc.scalar,
                     name=f"nei{b}") for b in range(BPC)]
        oho_sb = [ld(big, [128, 2, M], oh_orig[b], engine=nc.scalar,
                     name=f"oho{b}") for b in range(BPC)]

        with tc.tile_pool(name="psA", bufs=3, space="PSUM") as psA, \
             tc.tile_pool(name="psB", bufs=4, space="PSUM") as psB:

            # ================= phase A: context-entity hidden ================
            cehT = [keep.tile([128, BPC * EC], F32, name=f"cehT{k}")
                    for k in range(2)]
            for b in range(BPC):
                for k, src in ((0, cem[b]), (1, ceo[b])):
                    aps = psA.tile([128, EC], F32, space="PSUM", tag="a")
                    nc.tensor.matmul(out=aps[:], lhsT=src[:],
                                     rhs=oh1[:, b, :], start=True, stop=True)
                    nc.scalar.copy(out=cehT[k][:, b * EC:(b + 1) * EC],
                                   in_=aps[:])
            o1ps = psB.tile([128, BPC * EC], F32, space="PSUM", tag="b")
            nc.tensor.matmul(out=o1ps[:], lhsT=m1w[:, 0, :], rhs=cehT[0][:],
                             start=True, stop=False)
            nc.tensor.matmul(out=o1ps[:], lhsT=m1w[:, 1, :], rhs=cehT[1][:],
                             start=False, stop=True)
            o1T = work.tile([128, BPC * EC], F32)
            nc.scalar.activation(out=o1T[:], in_=o1ps[:],
                                 func=mybir.ActivationFunctionType.Relu,
                                 bias=m1b[:, 0:1])
            for h in range(2):
                tp = psA.tile([96, 128], F32, space="PSUM", tag="a")
                nc.tensor.transpose(out=tp[:], in_=o1T[:, h * 96:(h + 1) * 96],
                                    identity=ident[:])
                o1r = work.tile([96, 128], F32)
                nc.vector.tensor_copy(out=o1r[:], in_=tp[:])
                nc.sync.dma_start(out=out_ctx[h * 96:(h + 1) * 96, :],
                                  in_=o1r[:])

            # ================= phase B: entity attention + mlp2 ==============
            # early transposes: ceT + embT for all batches
            ceT = [keep.tile([128, 128], F32, name=f"ceT{b}")
                   for b in range(BPC)]
            embT = [keep.tile([128, N], F32, name=f"embT{b}")
                    for b in range(BPC)]
            for b in range(BPC):
                tp = psA.tile([128, 128], F32, space="PSUM", tag="a")
                nc.tensor.transpose(out=tp[:], in_=cem[b][:], identity=ident[:])
                nc.scalar.copy(out=ceT[b][:], in_=tp[:])
                for j in range(2):
                    tp2 = psA.tile([128, 128], F32, space="PSUM", tag="a")
                    nc.tensor.transpose(out=tp2[:], in_=embr[b][j][:],
                                        identity=ident[:])
                    nc.vector.tensor_copy(
                        out=embT[b][:, j * 128:(j + 1) * 128], in_=tp2[:])

            ehr = [[keep.tile([128, 128], F32, name=f"ehr{b}_{j}")
                    for j in range(2)] for b in range(BPC)]
            for b in range(BPC):
                # q_T = wq @ emb_T + bq
                qps = psB.tile([128, N], F32, space="PSUM", tag="b")
                nc.tensor.matmul(out=qps[:], lhsT=wq[:], rhs=embT[b][:],
                                 start=True, stop=True)
                qT = work.tile([128, N], F32)
                nc.scalar.activation(out=qT[:], in_=qps[:],
                                     func=mybir.ActivationFunctionType.Identity,
                                     bias=bqs[:, 0:1])

                awT = work.tile([128, N], F32)
                for ntl in range(2):
                    sps = psB.tile([128, 128], F32, space="PSUM", tag="b")
                    nc.tensor.matmul(out=sps[:],
                                     lhsT=qT[:, ntl * 128:(ntl + 1) * 128],
                                     rhs=ceT[b][:], start=True, stop=True)
                    sc = small.tile([128, 128], F32, name="sc")
                    nc.vector.tensor_tensor(out=sc[:], in0=sps[:],
                                            in1=amb[b][:],
                                            op=mybir.AluOpType.add)
                    rmax = small.tile([128, 1], F32, name="rmax")
                    nc.vector.tensor_reduce(out=rmax[:], in_=sc[:],
                                            axis=mybir.AxisListType.X,
                                            op=mybir.AluOpType.max,
                                            negate=True)
                    sexp = small.tile([128, 1], F32, name="sexp")
                    nc.scalar.activation(out=sc[:], in_=sc[:],
                                         func=mybir.ActivationFunctionType.Exp,
                                         bias=rmax[:, 0:1],
                                         accum_out=sexp[:, 0:1])
                    rs = small.tile([128, 1], F32, name="rs")
                    nc.vector.reciprocal(out=rs[:], in_=sexp[:])
                    nc.vector.tensor_scalar_mul(sc[:], sc[:], rs[:, 0:1])
                    tp3 = psA.tile([128, 128], F32, space="PSUM", tag="a")
                    nc.tensor.transpose(out=tp3[:], in_=sc[:],
                                        identity=ident[:])
                    nc.vector.tensor_copy(out=awT[:, ntl * 128:(ntl + 1) * 128],
                                          in_=tp3[:])

                # entity_align_T = ce_emb.T @ attw_T
                alps = psB.tile([128, N], F32, space="PSUM", tag="b")
                nc.tensor.matmul(out=alps[:], lhsT=cem[b][:], rhs=awT[:],
                                 start=True, stop=True)
                alT = work.tile([128, N], F32)
                nc.scalar.copy(out=alT[:], in_=alps[:])

                # entity_hidden_T = relu(mlp2_wT.T @ [emb_T; align_T] + b2)
                ehps = psB.tile([128, N], F32, space="PSUM", tag="b")
                nc.tensor.matmul(out=ehps[:], lhsT=m2w[:, 0, :], rhs=embT[b][:],
                                 start=True, stop=False)
                nc.tensor.matmul(out=ehps[:], lhsT=m2w[:, 1, :], rhs=alT[:],
                                 start=False, stop=True)
                ehT = work.tile([128, N], F32)
                nc.scalar.activation(out=ehT[:], in_=ehps[:],
                                     func=mybir.ActivationFunctionType.Relu,
                                     bias=m2b[:, 0:1])
                for j in range(2):
                    tp4 = psA.tile([128, 128], F32, space="PSUM", tag="a")
                    nc.tensor.transpose(out=tp4[:],
                                        in_=ehT[:, j * 128:(j + 1) * 128],
                                        identity=ident[:])
                    nc.vector.tensor_copy(out=ehr[b][j][:], in_=tp4[:])
                    nc.sync.dma_start(
                        out=eh_rows[b * N + j * 128:b * N + (j + 1) * 128, :],
                        in_=ehr[b][j][:])

        # ================= phase C: KB graph ============================
        kbiT = [keep.tile([128, M], F32, name=f"kbiT{b}") for b in range(BPC)]
        xsT = big.tile([128, nslot * SLOT], F32)

        with tc.tile_pool(name="psT", bufs=2, space="PSUM") as psT, \
             tc.tile_pool(name="psK", bufs=4, space="PSUM") as psK, \
             tc.tile_pool(name="psG", bufs=2, space="PSUM") as psG:

            # original-order kb_init via onehot matmul (mask folded in)
            for b in range(BPC):
                kps = psG.tile([128, M], F32, space="PSUM", tag="g")
                for kt in range(2):
                    nc.tensor.matmul(out=kps[:], lhsT=ehr[b][kt][:],
                                     rhs=oho_sb[b][:, kt, :],
                                     start=(kt == 0), stop=(kt == 1))
                nc.scalar.copy(out=kbiT[b][:], in_=kps[:])

            # sorted gather -> xs_T [128, nslot*32]
            for t in range(nt):
                g = small.tile([128, 128], F32, name="gat")
                nc.gpsimd.indirect_dma_start(
                    out=g[:], out_offset=None, in_=eh_rows[:],
                    in_offset=bass.IndirectOffsetOnAxis(
                        ap=sidx[:, t:t + 1], axis=0))
                nc.vector.tensor_scalar_mul(g[:], g[:], smsk[:, t:t + 1])
                tps = psT.tile([128, 128], F32, space="PSUM", tag="t")
                nc.tensor.transpose(out=tps[:], in_=g[:], identity=ident[:])
                nc.vector.tensor_copy(out=xsT[:, t * 128:(t + 1) * 128],
                                      in_=tps[:])

            # relation matvec: slot s -> sorted ks rows [32s, 32s+32)
            for t in range(nt):
                kp = psK.tile([128, 128], F32, space="PSUM", tag="k")
                for q in range(4):
                    s = 4 * t + q
                    ro = SLOT * q
                    nc.tensor.matmul(
                        out=kp[ro:ro + SLOT, :],
                        lhsT=xsT[:, s * SLOT:(s + 1) * SLOT],
                        rhs=wsel_sb[:, s, :],
                        start=True, stop=True,
                        tile_position=(0, ro))
                kr = small.tile([128, 128], F32, name="krow")
                if t % 2 == 0:
                    nc.scalar.copy(out=kr[:], in_=kp[:])
                else:
                    nc.vector.tensor_copy(out=kr[:], in_=kp[:])
                nc.sync.dma_start(out=ks_sort[t * 128:(t + 1) * 128, :],
                                  in_=kr[:])

        # aggregation (row-oriented): out rows = relu(
        #   kbi_T-slice.T @ W0_w.T + sum_kt nei_T-slice.T @ ks_rows[kt])
        with tc.tile_pool(name="psH", bufs=4, space="PSUM") as psH:
            for b in range(BPC):
                ksb = [small.tile([128, 128], F32, name="ksb", bufs=8)
                       for _ in range(4)]
                for kt in range(4):
                    nc.gpsimd.indirect_dma_start(
                        out=ksb[kt][:], out_offset=None, in_=ks_sort[:],
                        in_offset=bass.IndirectOffsetOnAxis(
                            ap=vidx[:, 4 * b + kt:4 * b + kt + 1], axis=0))
                for mt in range(4):
                    hps = psH.tile([128, 128], F32, space="PSUM", tag="h")
                    nc.tensor.matmul(
                        out=hps[:], lhsT=kbiT[b][:, mt * 128:(mt + 1) * 128],
                        rhs=w0[:], start=True, stop=False)
                    for kt in range(4):
                        nc.tensor.matmul(
                            out=hps[:],
                            lhsT=nei_sb[b][:, kt * M + mt * 128:
                                           kt * M + (mt + 1) * 128],
                            rhs=ksb[kt][:], start=False, stop=(kt == 3))
                    orow = small.tile([128, 128], F32, name="orow")
                    nc.scalar.activation(
                        out=orow[:], in_=hps[:],
                        func=mybir.ActivationFunctionType.Relu)
                    nc.sync.dma_start(
                        out=out_kb[b * M + mt * 128:b * M + (mt + 1) * 128, :],
                        in_=orow[:])

    nc.compile()
    return nc


_CACHE = {}


def _get_program(nslot, nt):
    key = (nslot, nt)
    if key not in _CACHE:
        _CACHE[key] = _build_program(nslot, nt)
    return _CACHE[key]


def kernel(**inputs):
    in_maps, nslot, nt = _host_prep(inputs)
    nc = _get_program(nslot, nt)
    res = run_bass_kernel_spmd(nc, in_maps, list(range(NCORES)))
    out_ctx = np.concatenate(
        [res.results[c]["out_ctx"].reshape(BPC, EC, 128) for c in range(NCORES)])
    out_kb = np.concatenate(
        [res.results[c]["out_kb"].reshape(BPC, M, 128) for c in range(NCORES)])
    return out_ctx, out_kb


# revision 8
# speedup vs baseline: 1.5881x; 1.1767x over previous
"""Trainium2 Bass kernel for nn_EntityEncoder (gnn_message_passing).

Contract: kernel(**inputs) takes the FULL unsharded inputs (numpy) and
returns the full outputs (context_entity_hidden [32,48,128],
kb_entity_hidden [32,512,128]) as a tuple, matching reference().

Strategy: data-parallel over the batch dim (4 batches per NeuronCore,
8 cores, one SPMD program). Gathers run on-device (indirect DMA /
onehot matmuls); the per-edge relation matvec uses a relation-sorted
32-wide slot layout with a per-core slot->weight table so the
instruction stream is identical on every core. kb_state rows are
stored sorted and un-sorted by an inverse-permutation gather feeding a
row-oriented PSUM-accumulated aggregation.
"""
import sys

sys.path.insert(0, "/opt/trn_rl_repo")

from contextlib import ExitStack

import numpy as np

import concourse.bass as bass
import concourse.tile as tile
from concourse import bacc, mybir
from concourse.bass_utils import run_bass_kernel_spmd
from concourse.masks import make_identity

# problem shapes (hardcoded per spec)
B, L, EC, N, M, D, R, V = 32, 128, 48, 256, 512, 128, 100, 40000
NCORES = 8
BPC = B // NCORES          # batches per core = 4
SLOT = 32                  # edges per matvec slot (PE col-group width)
EDG = BPC * M              # edges per core = 2048
NE = BPC * N               # entities per core = 1024
F32 = mybir.dt.float32
I32 = mybir.dt.int32


# ---------------------------------------------------------------- host prep

def _host_prep(inputs):
    ce_emb = np.asarray(inputs["context_emb"], np.float32)
    ce_out = np.asarray(inputs["context_outputs"], np.float32)
    cmask = np.asarray(inputs["context_mask"], np.int32)
    cpos = np.asarray(inputs["context_entity_pos"], np.int32)
    cemask = np.asarray(inputs["context_entity_mask"], np.int32)
    entity = np.asarray(inputs["entity"], np.int32)
    kbe = np.asarray(inputs["kb_entity"], np.int32)
    kbm = np.asarray(inputs["kb_entity_mask"], np.int32)
    kbc = np.asarray(inputs["kb_entity_col"], np.int32)
    nei = np.asarray(inputs["kb_entity_nei"], np.int32)
    embed_table = np.asarray(inputs["embed_table"], np.float32)
    mlp1_w = np.asarray(inputs["mlp1_w"], np.float32)
    mlp1_b = np.asarray(inputs["mlp1_b"], np.float32)
    mlp2_w = np.asarray(inputs["mlp2_w"], np.float32)
    mlp2_b = np.asarray(inputs["mlp2_b"], np.float32)
    attn_wq = np.asarray(inputs["attn_wq"], np.float32)
    attn_bq = np.asarray(inputs["attn_bq"], np.float32)
    W = np.asarray(inputs["W"], np.float32)
    W0_w = np.asarray(inputs["W0_w"], np.float32)

    W_T = np.ascontiguousarray(W.transpose(0, 2, 1))  # [R, j, i] = W[r][i, j]

    # per-core relation slotting: edges sorted by (rel, b, m), chunked to 32
    per_core = []
    nslot_need = 0
    for c in range(NCORES):
        sl = slice(BPC * c, BPC * (c + 1))
        rr = kbc[sl].ravel()
        bb = np.repeat(np.arange(BPC), M)
        mm = np.tile(np.arange(M), BPC)
        order = np.lexsort((mm, bb, rr))  # sorted by rr, then bb, then mm
        slots = []  # (rel, [edge flat ids b*M+m])
        i = 0
        while i < EDG:
            r = rr[order[i]]
            j = i
            while j < EDG and rr[order[j]] == r:
                j += 1
            for k in range(i, j, SLOT):
                slots.append((int(r), order[k:min(k + SLOT, j)]))
            i = j
        per_core.append((sl, slots))
        nslot_need = max(nslot_need, len(slots))
    nslot = -(-nslot_need // 4) * 4  # multiple of 4 -> whole 128-row tiles
    nt = nslot * SLOT // 128         # tiles in sorted layout
    nto = EDG // 128                 # tiles in original layout (16)

    # mlp*_w.T is [2D, D]; upload k-tiles as [128, 2, 128]
    m1 = mlp1_w.T.reshape(2, 128, 128).transpose(1, 0, 2)
    m2 = mlp2_w.T.reshape(2, 128, 128).transpose(1, 0, 2)
    # packed weights [128, 6, 128]: m1w0 m1w1 m2w0 m2w1 wq w0
    wpack = np.stack([m1[:, 0], m1[:, 1], m2[:, 0], m2[:, 1],
                      attn_wq.T, W0_w.T], axis=1)
    # packed biases [128, 3]: mlp1_b mlp2_b bq
    bpack = np.stack([mlp1_b, mlp2_b, attn_bq], axis=1)
    shared = dict(
        wpack=np.ascontiguousarray(wpack.astype(np.float32)),
        bpack=np.ascontiguousarray(bpack.astype(np.float32)),
        emb_tbl=embed_table,
    )

    in_maps = []
    for c in range(NCORES):
        sl, slots = per_core[c]
        am = np.where(cmask[sl] > 0, 0.0, -1e9).astype(np.float32)  # [4,128]
        amask = np.ascontiguousarray(
            np.broadcast_to(am[None], (128, BPC, 128)))  # pre-broadcast
        oh1 = np.zeros((BPC, L, EC), np.float32)
        for b in range(BPC):
            oh1[b, cpos[sl][b], np.arange(EC)] = cemask[sl][b].astype(np.float32)
        # sorted slot layout (gather indices + mask); inverse perm for unsort
        kbe_c, kbm_c = kbe[sl], kbm[sl]
        idx_s = np.zeros(nslot * SLOT, np.int32)
        msk_s = np.zeros(nslot * SLOT, np.float32)
        inv = np.zeros(EDG, np.int32)
        wsel = np.zeros((nslot, 128, 128), np.float32)
        for s, (r, edges) in enumerate(slots):
            wsel[s] = W_T[r]
            pos = SLOT * s + np.arange(len(edges))
            eb, em = edges // M, edges % M
            idx_s[pos] = eb * N + kbe_c[eb, em]
            msk_s[pos] = kbm_c[eb, em].astype(np.float32)
            inv[edges] = pos
        # packed int32 indices [128, 8 + nt + nto]: ent | sorted | inv
        ipack = np.ascontiguousarray(np.concatenate([
            entity[sl].ravel().reshape(NE // 128, 128).T.astype(np.int32),
            idx_s.reshape(nt, 128).T,
            inv.reshape(nto, 128).T], axis=1))
        msk_sorted = np.ascontiguousarray(msk_s.reshape(nt, 128).T)
        w_sel = np.ascontiguousarray(wsel.transpose(1, 0, 2))  # [128,nslot,128]

        # onehot for the original-order kb gather (mask folded in)
        oho = np.zeros((BPC, N, M), np.float32)
        for b in range(BPC):
            oho[b, kbe_c[b], np.arange(M)] = kbm_c[b].astype(np.float32)
        oh_orig = np.ascontiguousarray(
            oho.reshape(BPC, 2, 128, M).transpose(0, 2, 1, 3))  # [4,128,2,512]

        # degree-normalized transposed neighbor matrix, original order
        nei_c = nei[sl].astype(np.float32)            # [4, M, M]
        deg = np.clip(nei_c.sum(axis=2), 1.0, None)   # [4, M]
        nnT = (nei_c / deg[:, :, None]).transpose(0, 2, 1)  # [4, n, m]
        nei_t = np.ascontiguousarray(
            nnT.reshape(BPC, 4, 128, M).transpose(0, 2, 1, 3).reshape(
                BPC, 128, 4 * M))  # [4, 128, 2048]

        m = dict(shared)
        m.update(
            ce_emb=np.ascontiguousarray(ce_emb[sl]),
            ce_out=np.ascontiguousarray(ce_out[sl]),
            amask=amask,
            onehot1=np.ascontiguousarray(oh1.transpose(1, 0, 2)),  # [128,4,48]
            ipack=ipack,
            msk_sorted=msk_sorted,
            w_sel=w_sel,
            oh_orig=oh_orig,
            nei_t=nei_t,
        )
        in_maps.append(m)
    return in_maps, nslot, nt


# ------------------------------------------------------------- bass program

def _build_program(nslot, nt):
    nto = EDG // 128
    nc = bacc.Bacc("TRN2", target_bir_lowering=False, debug=False,
                   num_devices=NCORES)

    def din(name, shape, dt=F32):
        return nc.dram_tensor(name, list(shape), dt, kind="ExternalInput").ap()

    ce_emb = din("ce_emb", (BPC, 128, 128))
    ce_out = din("ce_out", (BPC, 128, 128))
    amask = din("amask", (128, BPC, 128))
    onehot1 = din("onehot1", (128, BPC, EC))
    wpack = din("wpack", (128, 6, 128))
    bpack = din("bpack", (128, 3))
    ipack = din("ipack", (128, NE // 128 + nt + nto), I32)
    msk_sorted = din("msk_sorted", (128, nt))
    emb_tbl = din("emb_tbl", (V, 128))
    w_sel = din("w_sel", (128, nslot, 128))
    oh_orig = din("oh_orig", (BPC, 128, 2, M))
    nei_t = din("nei_t", (BPC, 128, 4 * M))

    out_ctx = nc.dram_tensor("out_ctx", [BPC * EC, 128], F32,
                             kind="ExternalOutput").ap()
    out_kb = nc.dram_tensor("out_kb", [EDG, 128], F32,
                            kind="ExternalOutput").ap()

    eh_rows = nc.dram_tensor("eh_rows", [NE, 128], F32).ap()
    ks_sort = nc.dram_tensor("ks_sort", [nt * 128, 128], F32).ap()

    with tile.TileContext(nc) as tc, ExitStack() as ctx:
        consts = ctx.enter_context(tc.tile_pool(name="consts", bufs=1))
        big = ctx.enter_context(tc.tile_pool(name="big", bufs=1))
        work = ctx.enter_context(tc.tile_pool(name="work", bufs=3))
        keep = ctx.enter_context(tc.tile_pool(name="keep", bufs=1))
        small = ctx.enter_context(tc.tile_pool(name="small", bufs=4))

        # ---- gpsimd: identity first (all transposes need it)
        ident = consts.tile([128, 128], F32)
        make_identity(nc, ident[:])

        # ---- sync queue: small latency-critical loads, indices first
        def ld(pool, shape, src, dt=F32, name=None):
            t = pool.tile(shape, dt, name=name)
            nc.sync.dma_start(out=t[:], in_=src)
            return t

        ipk = ld(consts, [128, NE // 128 + nt + nto], ipack[:], I32, "ipk")
        eidx = ipk[:, 0:NE // 128]
        sidx = ipk[:, NE // 128:NE // 128 + nt]
        vidx = ipk[:, NE // 128 + nt:]
        wpk = ld(consts, [128, 6, 128], wpack[:], name="wpk")
        bpk = ld(consts, [128, 3], bpack[:], name="bpk")
        smsk = ld(consts, [128, nt], msk_sorted[:], name="smsk")
        oh1 = ld(consts, [128, BPC, EC], onehot1[:], name="oh1")
        ambc = ld(consts, [128, BPC, 128], amask[:], name="ambc")
        cem = [ld(consts, [128, 128], ce_emb[b], name=f"cem{b}")
               for b in range(BPC)]
        ceo = [ld(consts, [128, 128], ce_out[b], name=f"ceo{b}")
               for b in range(BPC)]

        # ---- gpsimd: embedding gathers early (phase B input)
        embr = [[keep.tile([128, 128], F32, name=f"embr{b}_{j}")
                 for j in range(2)] for b in range(BPC)]
        for b in range(BPC):
            for j in range(2):
                nc.gpsimd.indirect_dma_start(
                    out=embr[b][j][:], out_offset=None, in_=emb_tbl[:],
                    in_offset=bass.IndirectOffsetOnAxis(
                        ap=eidx[:, 2 * b + j:2 * b + j + 1], axis=0))

        # ---- bulk background loads, all on gpsimd (no compute role there)
        oho_sb = [big.tile([128, 2, M], F32, name=f"oho{b}") for b in range(BPC)]
        for b in range(BPC):
            nc.gpsimd.dma_start(out=oho_sb[b][:], in_=oh_orig[b])
        wsel_sb = big.tile([128, nslot, 128], F32)
        qs = nslot // 4
        for q in range(4):
            nc.gpsimd.dma_start(out=wsel_sb[:, q * qs:(q + 1) * qs, :],
                                in_=w_sel[:, q * qs:(q + 1) * qs, :])
        nei_sb = [big.tile([128, 4 * M], F32, name=f"nei{b}") for b in range(BPC)]
        for b in range(BPC):
            nc.gpsimd.dma_start(out=nei_sb[b][:], in_=nei_t[b])

        m1b, m2b, bqs = bpk[:, 0:1], bpk[:, 1:2], bpk[:, 2:3]

        with tc.tile_pool(name="psA", bufs=2, space="PSUM") as psA, \
             tc.tile_pool(name="psB", bufs=5, space="PSUM") as psB:

            # ================= phase A: context-entity hidden ================
            cehT = [keep.tile([128, BPC * EC], F32, name=f"cehT{k}")
                    for k in range(2)]
            for b in range(BPC):
                for k, src in ((0, cem[b]), (1, ceo[b])):
                    aps = psA.tile([128, EC], F32, space="PSUM", tag="a")
                    nc.tensor.matmul(out=aps[:], lhsT=src[:],
                                     rhs=oh1[:, b, :], start=True, stop=True)
                    nc.scalar.copy(out=cehT[k][:, b * EC:(b + 1) * EC],
                                   in_=aps[:])
            o1ps = psB.tile([128, BPC * EC], F32, space="PSUM", tag="b")
            nc.tensor.matmul(out=o1ps[:], lhsT=wpk[:, 0, :], rhs=cehT[0][:],
                             start=True, stop=False)
            nc.tensor.matmul(out=o1ps[:], lhsT=wpk[:, 1, :], rhs=cehT[1][:],
                             start=False, stop=True)
            o1T = work.tile([128, BPC * EC], F32)
            nc.scalar.activation(out=o1T[:], in_=o1ps[:],
                                 func=mybir.ActivationFunctionType.Relu,
                                 bias=m1b)
            for h in range(2):
                tp = psA.tile([96, 128], F32, space="PSUM", tag="a")
                nc.tensor.transpose(out=tp[:], in_=o1T[:, h * 96:(h + 1) * 96],
                                    identity=ident[:])
                o1r = work.tile([96, 128], F32)
                nc.vector.tensor_copy(out=o1r[:], in_=tp[:])
                nc.sync.dma_start(out=out_ctx[h * 96:(h + 1) * 96, :],
                                  in_=o1r[:])

            # ================= phase B: entity attention + mlp2 ==============
            # early transposes: ceT + embT for all batches
            ceT = [keep.tile([128, 128], F32, name=f"ceT{b}")
                   for b in range(BPC)]
            embT = keep.tile([128, BPC * N], F32)
            for b in range(BPC):
                tp = psA.tile([128, 128], F32, space="PSUM", tag="a")
                nc.tensor.transpose(out=tp[:], in_=cem[b][:], identity=ident[:])
                nc.scalar.copy(out=ceT[b][:], in_=tp[:])
                for j in range(2):
                    tp2 = psA.tile([128, 128], F32, space="PSUM", tag="a")
                    nc.tensor.transpose(out=tp2[:], in_=embr[b][j][:],
                                        identity=ident[:])
                    nc.vector.tensor_copy(
                        out=embT[:, b * N + j * 128:b * N + (j + 1) * 128],
                        in_=tp2[:])

            # q_T for all batches: 2 matmuls of [128, 512]
            qT = keep.tile([128, BPC * N], F32)
            for h in range(2):
                qps = psB.tile([128, 512], F32, space="PSUM", tag="b")
                nc.tensor.matmul(out=qps[:], lhsT=wpk[:, 4, :],
                                 rhs=embT[:, h * 512:(h + 1) * 512],
                                 start=True, stop=True)
                nc.scalar.activation(out=qT[:, h * 512:(h + 1) * 512],
                                     in_=qps[:],
                                     func=mybir.ActivationFunctionType.Identity,
                                     bias=bqs)

            ehr = [[keep.tile([128, 128], F32, name=f"ehr{b}_{j}")
                    for j in range(2)] for b in range(BPC)]
            alT = keep.tile([128, BPC * N], F32)
            for b in range(BPC):
                awT = work.tile([128, N], F32)
                for ntl in range(2):
                    sps = psB.tile([128, 128], F32, space="PSUM", tag="b")
                    nc.tensor.matmul(
                        out=sps[:],
                        lhsT=qT[:, b * N + ntl * 128:b * N + (ntl + 1) * 128],
                        rhs=ceT[b][:], start=True, stop=True)
                    sc = small.tile([128, 128], F32, name="sc")
                    nc.vector.tensor_tensor(out=sc[:], in0=sps[:],
                                            in1=ambc[:, b, :],
                                            op=mybir.AluOpType.add)
                    rmax = small.tile([128, 1], F32, name="rmax")
                    nc.vector.tensor_reduce(out=rmax[:], in_=sc[:],
                                            axis=mybir.AxisListType.X,
                                            op=mybir.AluOpType.max,
                                            negate=True)
                    sexp = small.tile([128, 1], F32, name="sexp")
                    nc.scalar.activation(out=sc[:], in_=sc[:],
                                         func=mybir.ActivationFunctionType.Exp,
                                         bias=rmax[:, 0:1],
                                         accum_out=sexp[:, 0:1])
                    rs = small.tile([128, 1], F32, name="rs")
                    nc.vector.reciprocal(out=rs[:], in_=sexp[:])
                    nc.vector.tensor_scalar_mul(sc[:], sc[:], rs[:, 0:1])
                    tp3 = psA.tile([128, 128], F32, space="PSUM", tag="a")
                    nc.tensor.transpose(out=tp3[:], in_=sc[:],
                                        identity=ident[:])
                    nc.vector.tensor_copy(out=awT[:, ntl * 128:(ntl + 1) * 128],
                                          in_=tp3[:])
                # entity_align_T = ce_emb.T @ attw_T
                alps = psB.tile([128, N], F32, space="PSUM", tag="b")
                nc.tensor.matmul(out=alps[:], lhsT=cem[b][:], rhs=awT[:],
                                 start=True, stop=True)
                nc.scalar.copy(out=alT[:, b * N:(b + 1) * N], in_=alps[:])

            # entity_hidden_T = relu(mlp2_wT.T @ [emb_T; align_T] + b2)
            for h in range(2):
                ehps = psB.tile([128, 512], F32, space="PSUM", tag="b")
                nc.tensor.matmul(out=ehps[:], lhsT=wpk[:, 2, :],
                                 rhs=embT[:, h * 512:(h + 1) * 512],
                                 start=True, stop=False)
                nc.tensor.matmul(out=ehps[:], lhsT=wpk[:, 3, :],
                                 rhs=alT[:, h * 512:(h + 1) * 512],
                                 start=False, stop=True)
                ehT = work.tile([128, 512], F32, name="ehT")
                nc.scalar.activation(out=ehT[:], in_=ehps[:],
                                     func=mybir.ActivationFunctionType.Relu,
                                     bias=m2b)
                for j in range(4):
                    b, jj = (h * 512 + j * 128) // N, ((h * 512 + j * 128) % N) // 128
                    tp4 = psA.tile([128, 128], F32, space="PSUM", tag="a")
                    nc.tensor.transpose(out=tp4[:],
                                        in_=ehT[:, j * 128:(j + 1) * 128],
                                        identity=ident[:])
                    nc.vector.tensor_copy(out=ehr[b][jj][:], in_=tp4[:])
                    nc.sync.dma_start(
                        out=eh_rows[b * N + jj * 128:b * N + (jj + 1) * 128, :],
                        in_=ehr[b][jj][:])

        # ================= phase C: KB graph ============================
        kbiT = [keep.tile([128, M], F32, name=f"kbiT{b}") for b in range(BPC)]
        xsT = big.tile([128, nslot * SLOT], F32)

        with tc.tile_pool(name="psT", bufs=2, space="PSUM") as psT, \
             tc.tile_pool(name="psK", bufs=4, space="PSUM") as psK, \
             tc.tile_pool(name="psG", bufs=2, space="PSUM") as psG:

            # original-order kb_init via onehot matmul (mask folded in)
            for b in range(BPC):
                kps = psG.tile([128, M], F32, space="PSUM", tag="g")
                for kt in range(2):
                    nc.tensor.matmul(out=kps[:], lhsT=ehr[b][kt][:],
                                     rhs=oho_sb[b][:, kt, :],
                                     start=(kt == 0), stop=(kt == 1))
                nc.scalar.copy(out=kbiT[b][:], in_=kps[:])

            # sorted gather -> xs_T [128, nslot*32]
            for t in range(nt):
                g = small.tile([128, 128], F32, name="gat")
                nc.gpsimd.indirect_dma_start(
                    out=g[:], out_offset=None, in_=eh_rows[:],
                    in_offset=bass.IndirectOffsetOnAxis(
                        ap=sidx[:, t:t + 1], axis=0))
                nc.vector.tensor_scalar_mul(g[:], g[:], smsk[:, t:t + 1])
                tps = psT.tile([128, 128], F32, space="PSUM", tag="t")
                nc.tensor.transpose(out=tps[:], in_=g[:], identity=ident[:])
                nc.vector.tensor_copy(out=xsT[:, t * 128:(t + 1) * 128],
                                      in_=tps[:])

            # relation matvec: slot s -> sorted ks rows [32s, 32s+32)
            for t in range(nt):
                kp = psK.tile([128, 128], F32, space="PSUM", tag="k")
                for q in range(4):
                    s = 4 * t + q
                    ro = SLOT * q
                    nc.tensor.matmul(
                        out=kp[ro:ro + SLOT, :],
                        lhsT=xsT[:, s * SLOT:(s + 1) * SLOT],
                        rhs=wsel_sb[:, s, :],
                        start=True, stop=True,
                        tile_position=(0, ro))
                kr = small.tile([128, 128], F32, name="krow")
                if t % 2 == 0:
                    nc.scalar.copy(out=kr[:], in_=kp[:])
                else:
                    nc.vector.tensor_copy(out=kr[:], in_=kp[:])
                nc.sync.dma_start(out=ks_sort[t * 128:(t + 1) * 128, :],
                                  in_=kr[:])

        # aggregation (row-oriented): out rows = relu(
        #   kbi_T-slice.T @ W0_w.T + sum_kt nei_T-slice.T @ ks_rows[kt])
        with tc.tile_pool(name="psH", bufs=4, space="PSUM") as psH:
            for b in range(BPC):
                ksb = [small.tile([128, 128], F32, name="ksb", bufs=8)
                       for _ in range(4)]
                for kt in range(4):
                    nc.gpsimd.indirect_dma_start(
                        out=ksb[kt][:], out_offset=None, in_=ks_sort[:],
                        in_offset=bass.IndirectOffsetOnAxis(
                            ap=vidx[:, 4 * b + kt:4 * b + kt + 1], axis=0))
                for mt in range(4):
                    hps = psH.tile([128, 128], F32, space="PSUM", tag="h")
                    nc.tensor.matmul(
                        out=hps[:], lhsT=kbiT[b][:, mt * 128:(mt + 1) * 128],
                        rhs=wpk[:, 5, :], start=True, stop=False)
                    for kt in range(4):
                        nc.tensor.matmul(
                            out=hps[:],
                            lhsT=nei_sb[b][:, kt * M + mt * 128:
                                           kt * M + (mt + 1) * 128],
                            rhs=ksb[kt][:], start=False, stop=(kt == 3))
                    orow = small.tile([128, 128], F32, name="orow")
                    nc.scalar.activation(
                        out=orow[:], in_=hps[:],
                        func=mybir.ActivationFunctionType.Relu)
                    nc.sync.dma_start(
                        out=out_kb[b * M + mt * 128:b * M + (mt + 1) * 128, :],
                        in_=orow[:])

    nc.compile()
    return nc


_CACHE = {}


def _get_program(nslot, nt):
    key = (nslot, nt)
    if key not in _CACHE:
        _CACHE[key] = _build_program(nslot, nt)
    return _CACHE[key]


def kernel(**inputs):
    in_maps, nslot, nt = _host_prep(inputs)
    nc = _get_program(nslot, nt)
    res = run_bass_kernel_spmd(nc, in_maps, list(range(NCORES)))
    out_ctx = np.concatenate(
        [res.results[c]["out_ctx"].reshape(BPC, EC, 128) for c in range(NCORES)])
    out_kb = np.concatenate(
        [res.results[c]["out_kb"].reshape(BPC, M, 128) for c in range(NCORES)])
    return out_ctx, out_kb
